# revision 18
# baseline (speedup 1.0000x reference)
"""Trainium2 Bass kernel for triplane SO3 deformable attention.

Sharding: data-parallel over batch (8 batches -> 8 cores). Each core
processes 2048 queries against its own triplane.

Device pipeline per core (per pair of 64-sample blocks):
  - dma_gather fp16 x-pair rows (512B) for center + 8 rotated anchors
  - DVE multiply by broadcast geometric weights (bilinear lerp factors)
  - PE "pair matrix" matmuls reduce (y, corners, planes, anchors) into PSUM
  - wsum = feat @ W_wf on PE; anchors weighted by wsum (on-device dependency)
  - final out = mixed @ (W_v@W_o) + feat via PE, DMA out

Host side only shards, relayouts planes (fp16, x-pair duplicated rows),
computes gather indices / lerp weights, and folds projection weights.
"""

import os
import sys

import numpy as np

sys.path.insert(0, "/opt/trn_rl_repo")

import ml_dtypes  # noqa: E402

import concourse.bacc as bacc  # noqa: E402
import concourse.bass as bass  # noqa: E402
import concourse.mybir as mybir  # noqa: E402
import concourse.tile as tile  # noqa: E402
from concourse import bass_utils  # noqa: E402
from concourse.library_config import mlp  # noqa: E402


def _install_ntff_hook():
    """Provide antenv.axon_hooks (absent in this image) so that
    run_bass_kernel_spmd(trace=True) can capture NTFF profiles via the
    axon PJRT .so. Mirrors trn_agent_boot/trn_boot.py step 6."""
    import types

    if "antenv.axon_hooks" in sys.modules:
        return True
    try:
        sys.path.insert(0, "/root/.axon_site/trn_agent_boot")
        import trn_boot  # noqa: E402

        hook = trn_boot._ntff_profile_via_ctypes("/opt/axon/libaxon_pjrt.so")
        if hook is None:
            return False
        mod = types.ModuleType("antenv.axon_hooks")
        mod._hook = hook
        mod.get_axon_ntff_profile_hook = lambda: mod._hook
        mod.set_axon_ntff_profile_hook = lambda h: setattr(mod, "_hook", h)
        sys.modules["antenv.axon_hooks"] = mod
        return True
    except Exception:
        return False

BS, NS, NCP, NH, C, HID, R = 8, 2048, 8, 8, 128, 128, 128
NBLK = NS // 64          # 32 blocks of 64 samples
NPAIR = NBLK // 2        # 16 pairs (128 samples each)
NCHUNK = 16              # mix gathers: 16 chunks of 2 blocks per plane
F16 = mybir.dt.float16
F32 = mybir.dt.float32
I16 = mybir.dt.int16

_CACHE = {}


def _wrap_idx(flat):
    """int16 flat index list -> [128, N/16] wrapped+replicated dma_gather layout."""
    n = flat.shape[0]
    w = flat.reshape(n // 16, 16).T.astype(np.int16)  # [16, N/16], elem j at [j%16, j//16]
    return np.tile(w, (8, 1))


def _host_prep(inputs):
    q = np.asarray(inputs["query_pos"], dtype=np.float32)      # (8, 2048, 9)
    planes = [np.asarray(inputs[k], dtype=np.float32)
              for k in ("plane_xz", "plane_xy", "plane_yz")]    # (8, C, R, R)
    cp = np.asarray(inputs["control_points"], dtype=np.float32)  # (8, 3)
    W_v = np.asarray(inputs["W_v"], dtype=np.float32)
    W_w = np.asarray(inputs["W_w"], dtype=np.float32)
    W_o = np.asarray(inputs["W_o"], dtype=np.float32)

    # folded projections
    W_wf = W_w.reshape(C, NCP, NH).sum(axis=1)                  # (C, 8)
    W_vo = W_v @ W_o                                            # (C, C)

    # rotation 6d -> matrix (rows b1,b2,b3), all fp32
    a1, a2 = q[..., 3:6], q[..., 6:9]
    b1 = a1 / np.linalg.norm(a1, axis=-1, keepdims=True)
    b2 = a2 - np.sum(b1 * a2, axis=-1, keepdims=True) * b1
    b2 = b2 / np.linalg.norm(b2, axis=-1, keepdims=True)
    b3 = np.cross(b1, b2)
    rot = np.stack([b1, b2, b3], axis=-2)                       # (8, 2048, 3, 3)
    cpr = np.einsum("bnpd,gd->bngp", rot, cp).astype(np.float32)  # (8, 2048, 8, 3)
    pts = np.concatenate([q[:, :, None, :3], q[:, :, None, :3] + cpr], axis=2)
    # (8, 2048, 9, 3); anchor 0 = center

    coord_pairs = [(0, 2), (0, 1), (1, 2)]  # (x-dim, y-dim) for xz, xy, yz

    # static device constants
    pairc = np.eye(128, dtype=np.float16)  # fp16 identity (PSUM accumulate)
    eye = np.eye(128, dtype=np.float32)

    xs = np.minimum(np.arange(R) + 1, R - 1)
    ys = np.minimum(np.arange(R) + 1, R - 1)

    core_inputs = []
    for b in range(BS):
        im = {"pairc": pairc, "eye": eye,
              "wwf": W_wf.astype(np.float32), "wvo": W_vo.astype(np.float32)}
        idxf_all, idxm_all, w4_all = [], [], []
        for pi in range(3):
            P = planes[pi][b]                       # (C, R, R)
            PT = np.transpose(P, (1, 2, 0))         # (y, x, c)
            E = np.concatenate(
                [PT, PT[:, xs, :], PT[ys, :, :], PT[ys][:, xs, :]],
                axis=-1)                            # (R, R, 4C) 2x2 patches
            im[f"ep{pi}"] = np.ascontiguousarray(
                E.reshape(R * R, 4 * C)).astype(np.float16)

            cx, cy = coord_pairs[pi]
            u = pts[b, :, :, cx]                    # (2048, 9)
            v = pts[b, :, :, cy]
            x = np.clip(u, 0.0, 1.0).astype(np.float32) * np.float32(R - 1)
            y = np.clip(v, 0.0, 1.0).astype(np.float32) * np.float32(R - 1)
            x0 = np.floor(x); y0 = np.floor(y)
            fx = (x - x0).astype(np.float32); fy = (y - y0).astype(np.float32)
            x0i = x0.astype(np.int32); y0i = y0.astype(np.int32)
            idx = y0i * R + x0i                    # (2048, 9) patch row id

            # corner weights (2048, 9, 4) order (y0x0, y0x1, y1x0, y1x1)
            wy = np.stack([1.0 - fy, fy], axis=-1)
            wx = np.stack([1.0 - fx, fx], axis=-1)
            w4 = (wy[..., :, None] * wx[..., None, :]).reshape(NS, 9, 4)
            w4_all.append(w4.astype(np.float32))

            # feat indices: anchor 0, order (pair, s2) -> partition = s2
            af = idx[:, 0].reshape(NPAIR, 128).ravel()
            idxf_all.append(_wrap_idx(af))
            # mix indices: anchors 1..8, order (pair, a, s2)
            am = idx[:, 1:].reshape(NPAIR, 128, 8).transpose(0, 2, 1).ravel()
            idxm_all.append(_wrap_idx(am))

        im["idxf"] = np.concatenate(idxf_all, axis=1)   # [128, 3*128]
        im["idxm"] = np.concatenate(idxm_all, axis=1)   # [128, 3*1024]

        W4 = np.stack(w4_all, axis=2)                   # (2048, 9, 3, 4) [s,a,p,cor]
        gf = W4[:, 0].reshape(NPAIR, 128, 3, 4).transpose(1, 0, 2, 3)
        im["gwf"] = np.ascontiguousarray(
            gf.reshape(128, NPAIR * 12)).astype(np.float16)  # (pair, p, cor)
        gm = W4[:, 1:].reshape(NPAIR, 128, 8, 3, 4).transpose(1, 0, 3, 2, 4)
        im["gwm"] = np.ascontiguousarray(
            gm.reshape(128, NPAIR * 96)).astype(np.float16)  # (pair, p, a, cor)
        core_inputs.append(im)
    return core_inputs


def _build():
    nc = bacc.Bacc("TRN2", target_bir_lowering=False, num_swdge_queues=4)
    ep = [nc.dram_tensor(f"ep{p}", [R * R, 4 * C], F16, kind="ExternalInput")
          for p in range(3)]
    idxf_d = nc.dram_tensor("idxf", [128, 3 * 128], I16, kind="ExternalInput")
    idxm_d = nc.dram_tensor("idxm", [128, 3 * 1024], I16, kind="ExternalInput")
    gwf_d = nc.dram_tensor("gwf", [128, NPAIR * 12], F16, kind="ExternalInput")
    gwm_d = nc.dram_tensor("gwm", [128, NPAIR * 96], F16, kind="ExternalInput")
    pairc_d = nc.dram_tensor("pairc", [128, 128], F16, kind="ExternalInput")
    wwf_d = nc.dram_tensor("wwf", [C, NCP], F32, kind="ExternalInput")
    wvo_d = nc.dram_tensor("wvo", [C, C], F32, kind="ExternalInput")
    eye_d = nc.dram_tensor("eye", [128, 128], F32, kind="ExternalInput")
    out_d = nc.dram_tensor("out", [NS, C], F32, kind="ExternalOutput")

    with tile.TileContext(nc) as tc:
        with (
            tc.tile_pool(name="const", bufs=1) as cpool,
            tc.tile_pool(name="gf", bufs=1) as gfpool,
            tc.tile_pool(name="gm", bufs=4) as gmpool,
            tc.tile_pool(name="ym", bufs=4) as ympool,
            tc.tile_pool(name="sm", bufs=3) as smpool,
            tc.tile_pool(name="pacc", bufs=2, space="PSUM") as paccpool,
            tc.tile_pool(name="pmix", bufs=2, space="PSUM") as pmixpool,
            tc.tile_pool(name="pmisc", bufs=4, space="PSUM") as pmiscpool,
        ):
            nc.gpsimd.load_library(mlp)

            def cload(name, dram, shape, dt):
                t = cpool.tile(shape, dt, tag=name)
                nc.sync.dma_start(t[:], dram[:])
                return t

            idxf_t = cload("idxf", idxf_d, [128, 3 * 128], I16)
            idxm_t = cload("idxm", idxm_d, [128, 3 * 1024], I16)
            gwf_t = cload("gwf", gwf_d, [128, NPAIR * 12], F16)
            gwm_t = cload("gwm", gwm_d, [128, NPAIR * 96], F16)
            ident_t = cload("pairc", pairc_d, [128, 128], F16)
            wwf_t = cload("wwf", wwf_d, [C, NCP], F32)
            wvo_t = cload("wvo", wvo_d, [C, C], F32)
            eye_t = cload("eye", eye_d, [128, 128], F32)

            # dma_gather crashes the exec unit above 1024 idx/call -> chunk
            qn = [0]

            def gather1k(dst, src_d, idx_t, col0, nidx):
                for h in range(nidx // 1024):
                    nc.gpsimd.dma_gather(
                        dst[:, h * 8:(h + 1) * 8, :], src_d[:],
                        idx_t[:, col0 + h * 64:col0 + (h + 1) * 64],
                        1024, 1024, 512, queue_num=qn[0] % 4)
                    qn[0] += 1

            # feat gathers: one patch row per sample: 2048 idx per plane
            gfeat = []
            for p in range(3):
                t = gfpool.tile([128, NPAIR, 512], F16, tag=f"gfe{p}")
                gather1k(t, ep[p], idxf_t, p * 128, 2048)
                gfeat.append(t)

            # mix gathers: per (pair, plane): 8 anchors * 128 samples = 1024
            gmix = {}
            for ch in range(NPAIR):
                for p in range(3):
                    t = gmpool.tile([128, 8, 512], F16, tag=f"gmx{p}")
                    gather1k(t, ep[p], idxm_t, p * 1024 + ch * 64, 1024)
                    gmix[(p, ch)] = t

            npair_run = int(os.environ.get("KPAIRS", str(NPAIR)))
            for pair in range(npair_run):
                # ---- feat: multiply + identity-accumulate reduce ----
                yfs = []
                for p in range(3):
                    yf = ympool.tile([128, 4, 128], F16, tag="yf")
                    in0 = gfeat[p][:, pair, :].rearrange(
                        "q (cor c) -> q cor c", cor=4)
                    gsl = gwf_t[:, pair * 12 + p * 4:pair * 12 + (p + 1) * 4]
                    in1 = gsl.unsqueeze(2).to_broadcast([128, 4, 128])
                    nc.vector.tensor_mul(yf[:], in0, in1)
                    yfs.append(yf)
                psF = paccpool.tile([128, 128], F32, tag="psF")
                n = 0
                for p in range(3):
                    for cor in range(4):
                        nc.tensor.matmul(
                            psF[:], ident_t[:], yfs[p][:, cor, :],
                            start=(n == 0), stop=(n == 11))
                        n += 1
                Fsb = smpool.tile([128, 128], F32, tag="Fsb")
                nc.scalar.copy(Fsb[:], psF[:])
                psFT = pmiscpool.tile([128, 128], F32, tag="psm")
                nc.tensor.transpose(psFT[:], Fsb[:], eye_t[:])
                FTsb = smpool.tile([128, 128], F32, tag="FTsb")
                nc.scalar.copy(FTsb[:], psFT[:])

                # ---- wsum (8 x s2) -> transpose -> per-sample anchor weights
                psW = pmiscpool.tile([128, 128], F32, tag="psm")
                nc.tensor.matmul(psW[:NCP, :], wwf_t[:], FTsb[:],
                                 start=True, stop=True)
                Wsb = smpool.tile([NCP, 128], F32, tag="Wsb")
                nc.scalar.copy(Wsb[:], psW[:NCP, :])
                psWT = pmiscpool.tile([128, 128], F32, tag="psm")
                nc.tensor.transpose(psWT[:, :NCP], Wsb[:], eye_t[:NCP, :NCP])
                WTsb = smpool.tile([128, NCP], F16, tag="WTsb")
                nc.scalar.copy(WTsb[:], psWT[:, :NCP])

                # alpha[s2, (p, a, cor)] = wsum[s2, a] * geo[s2, (p, a, cor)]
                al = smpool.tile([128, 96], F16, tag="al")
                in0 = gwm_t[:, pair * 96:(pair + 1) * 96].rearrange(
                    "q (p a cor) -> q p a cor", p=3, a=8)
                in1 = WTsb[:].unsqueeze(1).unsqueeze(3).to_broadcast(
                    [128, 3, 8, 4])
                nc.vector.tensor_mul(
                    al[:].rearrange("q (p a cor) -> q p a cor", p=3, a=8),
                    in0, in1)

                # ---- mixed: multiply + identity-accumulate reduce ----
                psM = pmixpool.tile([128, 128], F32, tag="psM")
                n = 0
                for p in range(3):
                    ym = ympool.tile([128, 32, 128], F16, tag="ym")
                    in0 = gmix[(p, pair)][:].rearrange(
                        "q a (cor c) -> q (a cor) c", cor=4)
                    in1 = al[:, p * 32:(p + 1) * 32].unsqueeze(
                        2).to_broadcast([128, 32, 128])
                    nc.vector.tensor_mul(ym[:], in0, in1)
                    for k in range(32):
                        nc.tensor.matmul(
                            psM[:], ident_t[:], ym[:, k, :],
                            start=(n == 0), stop=(n == 95))
                        n += 1

                # ---- final projection ----
                Msb = smpool.tile([128, 128], F32, tag="Msb")
                nc.scalar.copy(Msb[:], psM[:])
                psMT = pmiscpool.tile([128, 128], F32, tag="psm")
                nc.tensor.transpose(psMT[:], Msb[:], eye_t[:])
                MTsb = smpool.tile([128, 128], F32, tag="MTsb")
                nc.scalar.copy(MTsb[:], psMT[:])
                psO = pmiscpool.tile([128, 128], F32, tag="psm")
                nc.tensor.matmul(psO[:], wvo_t[:], MTsb[:], start=True, stop=False)
                nc.tensor.matmul(psO[:], eye_t[:], FTsb[:], start=False, stop=True)
                Osb = smpool.tile([128, 128], F32, tag="Osb")
                nc.scalar.copy(Osb[:], psO[:])
                psOT = pmiscpool.tile([128, 128], F32, tag="psm")
                nc.tensor.transpose(psOT[:], Osb[:], eye_t[:])
                OTsb = smpool.tile([128, 128], F32, tag="OTsb")
                nc.scalar.copy(OTsb[:], psOT[:])
                nc.sync.dma_start(out_d[pair * 128:(pair + 1) * 128, :], OTsb[:])
    nc.compile()
    return nc


def kernel(**inputs):
    core_inputs = _host_prep(inputs)
    if "nc" not in _CACHE:
        _CACHE["nc"] = _build()
    nc = _CACHE["nc"]
    trace = (os.environ.get("BASS_TRACE_KERNEL", "") not in ("", "0")
             and _install_ntff_hook())
    res = bass_utils.run_bass_kernel_spmd(
        nc, core_inputs, list(range(BS)), trace=trace)
    _CACHE["last_results"] = res
    outs = [np.asarray(res.results[i]["out"], dtype=np.float32)
            for i in range(BS)]
    return np.stack(outs, axis=0)


# revision 20
# speedup vs baseline: 1.0609x; 1.0609x over previous
"""Trainium2 Bass kernel for triplane SO3 deformable attention.

Sharding: data-parallel over batch (8 batches -> 8 cores). Each core
processes 2048 queries against its own triplane.

Device pipeline per core (per pair of 64-sample blocks):
  - dma_gather fp16 x-pair rows (512B) for center + 8 rotated anchors
  - DVE multiply by broadcast geometric weights (bilinear lerp factors)
  - PE "pair matrix" matmuls reduce (y, corners, planes, anchors) into PSUM
  - wsum = feat @ W_wf on PE; anchors weighted by wsum (on-device dependency)
  - final out = mixed @ (W_v@W_o) + feat via PE, DMA out

Host side only shards, relayouts planes (fp16, x-pair duplicated rows),
computes gather indices / lerp weights, and folds projection weights.
"""

import os
import sys

import numpy as np

sys.path.insert(0, "/opt/trn_rl_repo")

import ml_dtypes  # noqa: E402

import concourse.bacc as bacc  # noqa: E402
import concourse.bass as bass  # noqa: E402
import concourse.mybir as mybir  # noqa: E402
import concourse.tile as tile  # noqa: E402
from concourse import bass_utils  # noqa: E402
from concourse.library_config import mlp  # noqa: E402


def _install_ntff_hook():
    """Provide antenv.axon_hooks (absent in this image) so that
    run_bass_kernel_spmd(trace=True) can capture NTFF profiles via the
    axon PJRT .so. Mirrors trn_agent_boot/trn_boot.py step 6."""
    import types

    if "antenv.axon_hooks" in sys.modules:
        return True
    try:
        sys.path.insert(0, "/root/.axon_site/trn_agent_boot")
        import trn_boot  # noqa: E402

        hook = trn_boot._ntff_profile_via_ctypes("/opt/axon/libaxon_pjrt.so")
        if hook is None:
            return False
        mod = types.ModuleType("antenv.axon_hooks")
        mod._hook = hook
        mod.get_axon_ntff_profile_hook = lambda: mod._hook
        mod.set_axon_ntff_profile_hook = lambda h: setattr(mod, "_hook", h)
        sys.modules["antenv.axon_hooks"] = mod
        return True
    except Exception:
        return False

BS, NS, NCP, NH, C, HID, R = 8, 2048, 8, 8, 128, 128, 128
NBLK = NS // 64          # 32 blocks of 64 samples
NPAIR = NBLK // 2        # 16 pairs (128 samples each)
NCHUNK = 16              # mix gathers: 16 chunks of 2 blocks per plane
F16 = mybir.dt.float16
F32 = mybir.dt.float32
I16 = mybir.dt.int16

_CACHE = {}


def _wrap_idx(flat):
    """int16 flat index list -> [128, N/16] wrapped+replicated dma_gather layout."""
    n = flat.shape[0]
    w = flat.reshape(n // 16, 16).T.astype(np.int16)  # [16, N/16], elem j at [j%16, j//16]
    return np.tile(w, (8, 1))


def _host_prep(inputs):
    q = np.asarray(inputs["query_pos"], dtype=np.float32)      # (8, 2048, 9)
    planes = [np.asarray(inputs[k], dtype=np.float32)
              for k in ("plane_xz", "plane_xy", "plane_yz")]    # (8, C, R, R)
    cp = np.asarray(inputs["control_points"], dtype=np.float32)  # (8, 3)
    W_v = np.asarray(inputs["W_v"], dtype=np.float32)
    W_w = np.asarray(inputs["W_w"], dtype=np.float32)
    W_o = np.asarray(inputs["W_o"], dtype=np.float32)

    # folded projections
    W_wf = W_w.reshape(C, NCP, NH).sum(axis=1)                  # (C, 8)
    W_vo = W_v @ W_o                                            # (C, C)

    # rotation 6d -> matrix (rows b1,b2,b3), all fp32
    a1, a2 = q[..., 3:6], q[..., 6:9]
    b1 = a1 / np.linalg.norm(a1, axis=-1, keepdims=True)
    b2 = a2 - np.sum(b1 * a2, axis=-1, keepdims=True) * b1
    b2 = b2 / np.linalg.norm(b2, axis=-1, keepdims=True)
    b3 = np.cross(b1, b2)
    rot = np.stack([b1, b2, b3], axis=-2)                       # (8, 2048, 3, 3)
    cpr = np.einsum("bnpd,gd->bngp", rot, cp).astype(np.float32)  # (8, 2048, 8, 3)
    pts = np.concatenate([q[:, :, None, :3], q[:, :, None, :3] + cpr], axis=2)
    # (8, 2048, 9, 3); anchor 0 = center

    coord_pairs = [(0, 2), (0, 1), (1, 2)]  # (x-dim, y-dim) for xz, xy, yz

    # static device constants
    pairc = np.eye(128, dtype=np.float16)  # fp16 identity (PSUM accumulate)
    eye = np.eye(128, dtype=np.float32)

    xs = np.minimum(np.arange(R) + 1, R - 1)
    ys = np.minimum(np.arange(R) + 1, R - 1)

    core_inputs = []
    for b in range(BS):
        im = {"pairc": pairc, "eye": eye,
              "wwf": W_wf.astype(np.float32), "wvo": W_vo.astype(np.float32)}
        idxf_all, idxm_all, w4_all = [], [], []
        for pi in range(3):
            P = planes[pi][b]                       # (C, R, R)
            PT = np.transpose(P, (1, 2, 0))         # (y, x, c)
            E = np.concatenate(
                [PT, PT[:, xs, :], PT[ys, :, :], PT[ys][:, xs, :]],
                axis=-1)                            # (R, R, 4C) 2x2 patches
            im[f"ep{pi}"] = np.ascontiguousarray(
                E.reshape(R * R, 4 * C)).astype(np.float16)

            cx, cy = coord_pairs[pi]
            u = pts[b, :, :, cx]                    # (2048, 9)
            v = pts[b, :, :, cy]
            x = np.clip(u, 0.0, 1.0).astype(np.float32) * np.float32(R - 1)
            y = np.clip(v, 0.0, 1.0).astype(np.float32) * np.float32(R - 1)
            x0 = np.floor(x); y0 = np.floor(y)
            fx = (x - x0).astype(np.float32); fy = (y - y0).astype(np.float32)
            x0i = x0.astype(np.int32); y0i = y0.astype(np.int32)
            idx = y0i * R + x0i                    # (2048, 9) patch row id

            # corner weights (2048, 9, 4) order (y0x0, y0x1, y1x0, y1x1)
            wy = np.stack([1.0 - fy, fy], axis=-1)
            wx = np.stack([1.0 - fx, fx], axis=-1)
            w4 = (wy[..., :, None] * wx[..., None, :]).reshape(NS, 9, 4)
            w4_all.append(w4.astype(np.float32))

            # feat indices: anchor 0, order (pair, s2) -> partition = s2
            af = idx[:, 0].reshape(NPAIR, 128).ravel()
            idxf_all.append(_wrap_idx(af))
            # mix indices: anchors 1..8, order (pair, a, s2)
            am = idx[:, 1:].reshape(NPAIR, 128, 8).transpose(0, 2, 1).ravel()
            idxm_all.append(_wrap_idx(am))

        im["idxf"] = np.concatenate(idxf_all, axis=1)   # [128, 3*128]
        im["idxm"] = np.concatenate(idxm_all, axis=1)   # [128, 3*1024]

        W4 = np.stack(w4_all, axis=2)                   # (2048, 9, 3, 4) [s,a,p,cor]
        gf = W4[:, 0].reshape(NPAIR, 128, 3, 4).transpose(1, 0, 2, 3)
        im["gwf"] = np.ascontiguousarray(
            gf.reshape(128, NPAIR * 12)).astype(np.float16)  # (pair, p, cor)
        gm = W4[:, 1:].reshape(NPAIR, 128, 8, 3, 4).transpose(1, 0, 3, 2, 4)
        im["gwm"] = np.ascontiguousarray(
            gm.reshape(128, NPAIR * 96)).astype(np.float16)  # (pair, p, a, cor)
        core_inputs.append(im)
    return core_inputs


def _build():
    nc = bacc.Bacc("TRN2", target_bir_lowering=False, num_swdge_queues=4)
    ep = [nc.dram_tensor(f"ep{p}", [R * R, 4 * C], F16, kind="ExternalInput")
          for p in range(3)]
    idxf_d = nc.dram_tensor("idxf", [128, 3 * 128], I16, kind="ExternalInput")
    idxm_d = nc.dram_tensor("idxm", [128, 3 * 1024], I16, kind="ExternalInput")
    gwf_d = nc.dram_tensor("gwf", [128, NPAIR * 12], F16, kind="ExternalInput")
    gwm_d = nc.dram_tensor("gwm", [128, NPAIR * 96], F16, kind="ExternalInput")
    pairc_d = nc.dram_tensor("pairc", [128, 128], F16, kind="ExternalInput")
    wwf_d = nc.dram_tensor("wwf", [C, NCP], F32, kind="ExternalInput")
    wvo_d = nc.dram_tensor("wvo", [C, C], F32, kind="ExternalInput")
    eye_d = nc.dram_tensor("eye", [128, 128], F32, kind="ExternalInput")
    out_d = nc.dram_tensor("out", [NS, C], F32, kind="ExternalOutput")

    with tile.TileContext(nc) as tc:
        with (
            tc.tile_pool(name="const", bufs=1) as cpool,
            tc.tile_pool(name="gf", bufs=1) as gfpool,
            tc.tile_pool(name="gm", bufs=3) as gmpool,
            tc.tile_pool(name="ym", bufs=3) as ympool,
            tc.tile_pool(name="sm", bufs=3) as smpool,
            tc.tile_pool(name="pacc", bufs=2, space="PSUM") as paccpool,
            tc.tile_pool(name="pmix", bufs=2, space="PSUM") as pmixpool,
            tc.tile_pool(name="pmisc", bufs=4, space="PSUM") as pmiscpool,
        ):
            nc.gpsimd.load_library(mlp)

            def cload(name, dram, shape, dt):
                t = cpool.tile(shape, dt, tag=name)
                nc.sync.dma_start(t[:], dram[:])
                return t

            idxf_t = cload("idxf", idxf_d, [128, 3 * 128], I16)
            idxm_t = cload("idxm", idxm_d, [128, 3 * 1024], I16)
            gwf_t = cload("gwf", gwf_d, [128, NPAIR * 12], F16)
            gwm_t = cload("gwm", gwm_d, [128, NPAIR * 96], F16)
            ident_t = cload("pairc", pairc_d, [128, 128], F16)
            wwf_t = cload("wwf", wwf_d, [C, NCP], F32)
            wvo_t = cload("wvo", wvo_d, [C, C], F32)
            eye_t = cload("eye", eye_d, [128, 128], F32)

            # dma_gather crashes the exec unit above 1024 idx/call -> chunk
            qn = [0]

            def gather1k(dst, src_d, idx_t, col0, nidx):
                for h in range(nidx // 1024):
                    nc.gpsimd.dma_gather(
                        dst[:, h * 8:(h + 1) * 8, :], src_d[:],
                        idx_t[:, col0 + h * 64:col0 + (h + 1) * 64],
                        1024, 1024, 512, queue_num=qn[0] % 4)
                    qn[0] += 1

            # feat gathers: one patch row per sample: 2048 idx per plane
            gfeat = []
            for p in range(3):
                t = gfpool.tile([128, NPAIR, 512], F16, tag=f"gfe{p}")
                gather1k(t, ep[p], idxf_t, p * 128, 2048)
                gfeat.append(t)

            # mix gathers: per (pair, plane): 8 anchors * 128 samples = 1024
            gmix = {}
            for ch in range(NPAIR):
                for p in range(3):
                    t = gmpool.tile([128, 8, 512], F16, tag=f"gmx{p}")
                    gather1k(t, ep[p], idxm_t, p * 1024 + ch * 64, 1024)
                    gmix[(p, ch)] = t

            npair_run = int(os.environ.get("KPAIRS", str(NPAIR)))
            for pair in range(npair_run):
                # ---- feat: multiply + identity-accumulate reduce ----
                yfs = []
                for p in range(3):
                    yf = ympool.tile([128, 4, 128], F16, tag="yf")
                    in0 = gfeat[p][:, pair, :].rearrange(
                        "q (cor c) -> q cor c", cor=4)
                    gsl = gwf_t[:, pair * 12 + p * 4:pair * 12 + (p + 1) * 4]
                    in1 = gsl.unsqueeze(2).to_broadcast([128, 4, 128])
                    nc.vector.tensor_mul(yf[:], in0, in1)
                    yfs.append(yf)
                psF = paccpool.tile([128, 128], F32, tag="psF")
                n = 0
                for p in range(3):
                    for cor in range(4):
                        nc.tensor.matmul(
                            psF[:], ident_t[:], yfs[p][:, cor, :],
                            start=(n == 0), stop=(n == 11))
                        n += 1
                Fsb = smpool.tile([128, 128], F32, tag="Fsb")
                nc.scalar.copy(Fsb[:], psF[:])
                psFT = pmiscpool.tile([128, 128], F32, tag="psm")
                nc.tensor.transpose(psFT[:], Fsb[:], eye_t[:])
                FTsb = smpool.tile([128, 128], F32, tag="FTsb")
                nc.scalar.copy(FTsb[:], psFT[:])

                # ---- wsum (8 x s2) -> transpose -> per-sample anchor weights
                psW = pmiscpool.tile([128, 128], F32, tag="psm")
                nc.tensor.matmul(psW[:NCP, :], wwf_t[:], FTsb[:],
                                 start=True, stop=True)
                Wsb = smpool.tile([NCP, 128], F32, tag="Wsb")
                nc.scalar.copy(Wsb[:], psW[:NCP, :])
                psWT = pmiscpool.tile([128, 128], F32, tag="psm")
                nc.tensor.transpose(psWT[:, :NCP], Wsb[:], eye_t[:NCP, :NCP])
                WTsb = smpool.tile([128, NCP], F16, tag="WTsb")
                nc.scalar.copy(WTsb[:], psWT[:, :NCP])

                # alpha[s2, (p, a, cor)] = wsum[s2, a] * geo[s2, (p, a, cor)]
                al = smpool.tile([128, 96], F16, tag="al")
                in0 = gwm_t[:, pair * 96:(pair + 1) * 96].rearrange(
                    "q (p a cor) -> q p a cor", p=3, a=8)
                in1 = WTsb[:].unsqueeze(1).unsqueeze(3).to_broadcast(
                    [128, 3, 8, 4])
                nc.vector.tensor_mul(
                    al[:].rearrange("q (p a cor) -> q p a cor", p=3, a=8),
                    in0, in1)

                # ---- mixed: multiply + identity-accumulate reduce ----
                psM = pmixpool.tile([128, 128], F32, tag="psM")
                n = 0
                for p in range(3):
                    # expand alpha on ScalarE so the DVE multiply stays
                    # step-1 on both operands (2x mode)
                    alx = ympool.tile([128, 32, 128], F16, tag="alx")
                    in1 = al[:, p * 32:(p + 1) * 32].unsqueeze(
                        2).to_broadcast([128, 32, 128])
                    nc.scalar.copy(alx[:], in1)
                    ym = ympool.tile([128, 32, 128], F16, tag="ym")
                    in0 = gmix[(p, pair)][:].rearrange(
                        "q a (cor c) -> q (a cor) c", cor=4)
                    nc.vector.tensor_mul(ym[:], in0, alx[:])
                    for k in range(32):
                        nc.tensor.matmul(
                            psM[:], ident_t[:], ym[:, k, :],
                            start=(n == 0), stop=(n == 95))
                        n += 1

                # ---- final projection ----
                Msb = smpool.tile([128, 128], F32, tag="Msb")
                nc.scalar.copy(Msb[:], psM[:])
                psMT = pmiscpool.tile([128, 128], F32, tag="psm")
                nc.tensor.transpose(psMT[:], Msb[:], eye_t[:])
                MTsb = smpool.tile([128, 128], F32, tag="MTsb")
                nc.scalar.copy(MTsb[:], psMT[:])
                psO = pmiscpool.tile([128, 128], F32, tag="psm")
                nc.tensor.matmul(psO[:], wvo_t[:], MTsb[:], start=True, stop=False)
                nc.tensor.matmul(psO[:], eye_t[:], FTsb[:], start=False, stop=True)
                Osb = smpool.tile([128, 128], F32, tag="Osb")
                nc.scalar.copy(Osb[:], psO[:])
                psOT = pmiscpool.tile([128, 128], F32, tag="psm")
                nc.tensor.transpose(psOT[:], Osb[:], eye_t[:])
                OTsb = smpool.tile([128, 128], F32, tag="OTsb")
                nc.scalar.copy(OTsb[:], psOT[:])
                nc.sync.dma_start(out_d[pair * 128:(pair + 1) * 128, :], OTsb[:])
    nc.compile()
    return nc


def kernel(**inputs):
    core_inputs = _host_prep(inputs)
    if "nc" not in _CACHE:
        _CACHE["nc"] = _build()
    nc = _CACHE["nc"]
    trace = (os.environ.get("BASS_TRACE_KERNEL", "") not in ("", "0")
             and _install_ntff_hook())
    res = bass_utils.run_bass_kernel_spmd(
        nc, core_inputs, list(range(BS)), trace=trace)
    _CACHE["last_results"] = res
    outs = [np.asarray(res.results[i]["out"], dtype=np.float32)
            for i in range(BS)]
    return np.stack(outs, axis=0)


# revision 21
# speedup vs baseline: 1.4303x; 1.3481x over previous
"""Trainium2 Bass kernel for triplane SO3 deformable attention.

Sharding: data-parallel over batch (8 batches -> 8 cores). Each core
processes 2048 queries against its own triplane.

Device pipeline per core (per pair of 64-sample blocks):
  - dma_gather fp16 x-pair rows (512B) for center + 8 rotated anchors
  - DVE multiply by broadcast geometric weights (bilinear lerp factors)
  - PE "pair matrix" matmuls reduce (y, corners, planes, anchors) into PSUM
  - wsum = feat @ W_wf on PE; anchors weighted by wsum (on-device dependency)
  - final out = mixed @ (W_v@W_o) + feat via PE, DMA out

Host side only shards, relayouts planes (fp16, x-pair duplicated rows),
computes gather indices / lerp weights, and folds projection weights.
"""

import os
import sys

import numpy as np

sys.path.insert(0, "/opt/trn_rl_repo")

import ml_dtypes  # noqa: E402

import concourse.bacc as bacc  # noqa: E402
import concourse.bass as bass  # noqa: E402
import concourse.mybir as mybir  # noqa: E402
import concourse.tile as tile  # noqa: E402
from concourse import bass_utils  # noqa: E402
from concourse.library_config import mlp  # noqa: E402


def _install_ntff_hook():
    """Provide antenv.axon_hooks (absent in this image) so that
    run_bass_kernel_spmd(trace=True) can capture NTFF profiles via the
    axon PJRT .so. Mirrors trn_agent_boot/trn_boot.py step 6."""
    import types

    if "antenv.axon_hooks" in sys.modules:
        return True
    try:
        sys.path.insert(0, "/root/.axon_site/trn_agent_boot")
        import trn_boot  # noqa: E402

        hook = trn_boot._ntff_profile_via_ctypes("/opt/axon/libaxon_pjrt.so")
        if hook is None:
            return False
        mod = types.ModuleType("antenv.axon_hooks")
        mod._hook = hook
        mod.get_axon_ntff_profile_hook = lambda: mod._hook
        mod.set_axon_ntff_profile_hook = lambda h: setattr(mod, "_hook", h)
        sys.modules["antenv.axon_hooks"] = mod
        return True
    except Exception:
        return False

BS, NS, NCP, NH, C, HID, R = 8, 2048, 8, 8, 128, 128, 128
NBLK = NS // 64          # 32 blocks of 64 samples
NPAIR = NBLK // 2        # 16 pairs (128 samples each)
NCHUNK = 16              # mix gathers: 16 chunks of 2 blocks per plane
F16 = mybir.dt.float16
F32 = mybir.dt.float32
I16 = mybir.dt.int16

_CACHE = {}


def _wrap_idx(flat):
    """int16 flat index list -> [128, N/16] wrapped+replicated dma_gather layout."""
    n = flat.shape[0]
    w = flat.reshape(n // 16, 16).T.astype(np.int16)  # [16, N/16], elem j at [j%16, j//16]
    return np.tile(w, (8, 1))


def _host_prep(inputs):
    q = np.asarray(inputs["query_pos"], dtype=np.float32)      # (8, 2048, 9)
    planes = [np.asarray(inputs[k], dtype=np.float32)
              for k in ("plane_xz", "plane_xy", "plane_yz")]    # (8, C, R, R)
    cp = np.asarray(inputs["control_points"], dtype=np.float32)  # (8, 3)
    W_v = np.asarray(inputs["W_v"], dtype=np.float32)
    W_w = np.asarray(inputs["W_w"], dtype=np.float32)
    W_o = np.asarray(inputs["W_o"], dtype=np.float32)

    # folded projections
    W_wf = W_w.reshape(C, NCP, NH).sum(axis=1)                  # (C, 8)
    W_vo = W_v @ W_o                                            # (C, C)

    # rotation 6d -> matrix (rows b1,b2,b3), all fp32
    a1, a2 = q[..., 3:6], q[..., 6:9]
    b1 = a1 / np.linalg.norm(a1, axis=-1, keepdims=True)
    b2 = a2 - np.sum(b1 * a2, axis=-1, keepdims=True) * b1
    b2 = b2 / np.linalg.norm(b2, axis=-1, keepdims=True)
    b3 = np.cross(b1, b2)
    rot = np.stack([b1, b2, b3], axis=-2)                       # (8, 2048, 3, 3)
    cpr = np.einsum("bnpd,gd->bngp", rot, cp).astype(np.float32)  # (8, 2048, 8, 3)
    pts = np.concatenate([q[:, :, None, :3], q[:, :, None, :3] + cpr], axis=2)
    # (8, 2048, 9, 3); anchor 0 = center

    coord_pairs = [(0, 2), (0, 1), (1, 2)]  # (x-dim, y-dim) for xz, xy, yz

    # static device constants
    pairc = np.eye(128, dtype=np.float16)  # fp16 identity (PSUM accumulate)
    eye = np.eye(128, dtype=np.float32)

    xs = np.minimum(np.arange(R) + 1, R - 1)
    ys = np.minimum(np.arange(R) + 1, R - 1)

    core_inputs = []
    for b in range(BS):
        im = {"pairc": pairc, "eye": eye,
              "wwf": W_wf.astype(np.float32), "wvo": W_vo.astype(np.float32)}
        idxf_all, idxm_all, w4_all = [], [], []
        for pi in range(3):
            P = planes[pi][b]                       # (C, R, R)
            PT = np.transpose(P, (1, 2, 0))         # (y, x, c)
            E = np.concatenate(
                [PT, PT[:, xs, :], PT[ys, :, :], PT[ys][:, xs, :]],
                axis=-1)                            # (R, R, 4C) 2x2 patches
            im[f"ep{pi}"] = np.ascontiguousarray(
                E.reshape(R * R, 4 * C)).astype(np.float16)

            cx, cy = coord_pairs[pi]
            u = pts[b, :, :, cx]                    # (2048, 9)
            v = pts[b, :, :, cy]
            x = np.clip(u, 0.0, 1.0).astype(np.float32) * np.float32(R - 1)
            y = np.clip(v, 0.0, 1.0).astype(np.float32) * np.float32(R - 1)
            x0 = np.floor(x); y0 = np.floor(y)
            fx = (x - x0).astype(np.float32); fy = (y - y0).astype(np.float32)
            x0i = x0.astype(np.int32); y0i = y0.astype(np.int32)
            idx = y0i * R + x0i                    # (2048, 9) patch row id

            # corner weights (2048, 9, 4) order (y0x0, y0x1, y1x0, y1x1)
            wy = np.stack([1.0 - fy, fy], axis=-1)
            wx = np.stack([1.0 - fx, fx], axis=-1)
            w4 = (wy[..., :, None] * wx[..., None, :]).reshape(NS, 9, 4)
            w4_all.append(w4.astype(np.float32))

            # feat indices: anchor 0, order (pair, s2) -> partition = s2
            af = idx[:, 0].reshape(NPAIR, 128).ravel()
            idxf_all.append(_wrap_idx(af))
            # mix indices: anchors 1..8, order (pair, a, s2)
            am = idx[:, 1:].reshape(NPAIR, 128, 8).transpose(0, 2, 1).ravel()
            idxm_all.append(_wrap_idx(am))

        im["idxf"] = np.concatenate(idxf_all, axis=1)   # [128, 3*128]
        im["idxm"] = np.concatenate(idxm_all, axis=1)   # [128, 3*1024]

        W4 = np.stack(w4_all, axis=2)                   # (2048, 9, 3, 4) [s,a,p,cor]
        gf = W4[:, 0].reshape(NPAIR, 128, 3, 4).transpose(1, 0, 2, 3)
        im["gwf"] = np.ascontiguousarray(
            gf.reshape(128, NPAIR * 12)).astype(np.float16)  # (pair, p, cor)
        gm = W4[:, 1:].reshape(NPAIR, 128, 8, 3, 4).transpose(1, 0, 3, 2, 4)
        im["gwm"] = np.ascontiguousarray(
            gm.reshape(128, NPAIR * 96)).astype(np.float16)  # (pair, p, a, cor)
        core_inputs.append(im)
    return core_inputs


def _build():
    nc = bacc.Bacc("TRN2", target_bir_lowering=False, num_swdge_queues=4)
    ep = [nc.dram_tensor(f"ep{p}", [R * R, 4 * C], F16, kind="ExternalInput")
          for p in range(3)]
    idxf_d = nc.dram_tensor("idxf", [128, 3 * 128], I16, kind="ExternalInput")
    idxm_d = nc.dram_tensor("idxm", [128, 3 * 1024], I16, kind="ExternalInput")
    gwf_d = nc.dram_tensor("gwf", [128, NPAIR * 12], F16, kind="ExternalInput")
    gwm_d = nc.dram_tensor("gwm", [128, NPAIR * 96], F16, kind="ExternalInput")
    pairc_d = nc.dram_tensor("pairc", [128, 128], F16, kind="ExternalInput")
    wwf_d = nc.dram_tensor("wwf", [C, NCP], F32, kind="ExternalInput")
    wvo_d = nc.dram_tensor("wvo", [C, C], F32, kind="ExternalInput")
    eye_d = nc.dram_tensor("eye", [128, 128], F32, kind="ExternalInput")
    out_d = nc.dram_tensor("out", [NS, C], F32, kind="ExternalOutput")

    with tile.TileContext(nc) as tc:
        with (
            tc.tile_pool(name="const", bufs=1) as cpool,
            tc.tile_pool(name="gf", bufs=1) as gfpool,
            tc.tile_pool(name="gm", bufs=3) as gmpool,
            tc.tile_pool(name="ym", bufs=3) as ympool,
            tc.tile_pool(name="sm", bufs=3) as smpool,
            tc.tile_pool(name="ft", bufs=NPAIR) as ftpool,
            tc.tile_pool(name="alp", bufs=NPAIR) as alpool,
            tc.tile_pool(name="pacc", bufs=2, space="PSUM") as paccpool,
            tc.tile_pool(name="pmix", bufs=2, space="PSUM") as pmixpool,
            tc.tile_pool(name="pmisc", bufs=4, space="PSUM") as pmiscpool,
        ):
            nc.gpsimd.load_library(mlp)

            def cload(name, dram, shape, dt):
                t = cpool.tile(shape, dt, tag=name)
                nc.sync.dma_start(t[:], dram[:])
                return t

            idxf_t = cload("idxf", idxf_d, [128, 3 * 128], I16)
            idxm_t = cload("idxm", idxm_d, [128, 3 * 1024], I16)
            gwf_t = cload("gwf", gwf_d, [128, NPAIR * 12], F16)
            gwm_t = cload("gwm", gwm_d, [128, NPAIR * 96], F16)
            ident_t = cload("pairc", pairc_d, [128, 128], F16)
            wwf_t = cload("wwf", wwf_d, [C, NCP], F32)
            wvo_t = cload("wvo", wvo_d, [C, C], F32)
            eye_t = cload("eye", eye_d, [128, 128], F32)

            # dma_gather crashes the exec unit above 1024 idx/call -> chunk
            qn = [0]

            def gather1k(dst, src_d, idx_t, col0, nidx):
                for h in range(nidx // 1024):
                    nc.gpsimd.dma_gather(
                        dst[:, h * 8:(h + 1) * 8, :], src_d[:],
                        idx_t[:, col0 + h * 64:col0 + (h + 1) * 64],
                        1024, 1024, 512, queue_num=qn[0] % 4)
                    qn[0] += 1

            # feat gathers: one patch row per sample: 2048 idx per plane
            gfeat = []
            for p in range(3):
                t = gfpool.tile([128, NPAIR, 512], F16, tag=f"gfe{p}")
                gather1k(t, ep[p], idxf_t, p * 128, 2048)
                gfeat.append(t)

            # mix gathers: per (pair, plane): 8 anchors * 128 samples = 1024
            gmix = {}
            for ch in range(NPAIR):
                for p in range(3):
                    t = gmpool.tile([128, 8, 512], F16, tag=f"gmx{p}")
                    gather1k(t, ep[p], idxm_t, p * 1024 + ch * 64, 1024)
                    gmix[(p, ch)] = t

            npair_run = int(os.environ.get("KPAIRS", str(NPAIR)))
            fts, als = {}, {}
            # ---- phase 1: feat -> wsum -> alpha for every pair (only
            # needs the small feat gathers, so it all runs early) ----
            for pair in range(npair_run):
                yfs = []
                for p in range(3):
                    yf = ympool.tile([128, 4, 128], F16, tag="yf")
                    in0 = gfeat[p][:, pair, :].rearrange(
                        "q (cor c) -> q cor c", cor=4)
                    gsl = gwf_t[:, pair * 12 + p * 4:pair * 12 + (p + 1) * 4]
                    in1 = gsl.unsqueeze(2).to_broadcast([128, 4, 128])
                    nc.vector.tensor_mul(yf[:], in0, in1)
                    yfs.append(yf)
                psF = paccpool.tile([128, 128], F32, tag="psF")
                n = 0
                for p in range(3):
                    for cor in range(4):
                        nc.tensor.matmul(
                            psF[:], ident_t[:], yfs[p][:, cor, :],
                            start=(n == 0), stop=(n == 11))
                        n += 1
                Fsb = smpool.tile([128, 128], F32, tag="Fsb")
                nc.scalar.copy(Fsb[:], psF[:])
                psFT = pmiscpool.tile([128, 128], F32, tag="psm")
                nc.tensor.transpose(psFT[:], Fsb[:], eye_t[:])
                FTsb = ftpool.tile([128, 128], F32, tag="FTsb")
                nc.scalar.copy(FTsb[:], psFT[:])
                psW = pmiscpool.tile([128, 128], F32, tag="psm")
                nc.tensor.matmul(psW[:NCP, :], wwf_t[:], FTsb[:],
                                 start=True, stop=True)
                Wsb = smpool.tile([NCP, 128], F32, tag="Wsb")
                nc.scalar.copy(Wsb[:], psW[:NCP, :])
                psWT = pmiscpool.tile([128, 128], F32, tag="psm")
                nc.tensor.transpose(psWT[:, :NCP], Wsb[:], eye_t[:NCP, :NCP])
                WTsb = smpool.tile([128, NCP], F16, tag="WTsb")
                nc.scalar.copy(WTsb[:], psWT[:, :NCP])
                al = alpool.tile([128, 96], F16, tag="al")
                in0 = gwm_t[:, pair * 96:(pair + 1) * 96].rearrange(
                    "q (p a cor) -> q p a cor", p=3, a=8)
                in1 = WTsb[:].unsqueeze(1).unsqueeze(3).to_broadcast(
                    [128, 3, 8, 4])
                nc.vector.tensor_mul(
                    al[:].rearrange("q (p a cor) -> q p a cor", p=3, a=8),
                    in0, in1)
                fts[pair], als[pair] = FTsb, al

            # ---- phase 2: mix multiply + reduce + projection (short
            # tail behind each pair's mix gather) ----
            for pair in range(npair_run):
                FTsb, al = fts[pair], als[pair]
                psM = pmixpool.tile([128, 128], F32, tag="psM")
                n = 0
                for p in range(3):
                    alx = ympool.tile([128, 32, 128], F16, tag="alx")
                    in1 = al[:, p * 32:(p + 1) * 32].unsqueeze(
                        2).to_broadcast([128, 32, 128])
                    nc.scalar.copy(alx[:], in1)
                    ym = ympool.tile([128, 32, 128], F16, tag="ym")
                    in0 = gmix[(p, pair)][:].rearrange(
                        "q a (cor c) -> q (a cor) c", cor=4)
                    nc.vector.tensor_mul(ym[:], in0, alx[:])
                    for k in range(32):
                        nc.tensor.matmul(
                            psM[:], ident_t[:], ym[:, k, :],
                            start=(n == 0), stop=(n == 95))
                        n += 1
                Msb = smpool.tile([128, 128], F32, tag="Msb")
                nc.scalar.copy(Msb[:], psM[:])
                psMT = pmiscpool.tile([128, 128], F32, tag="psm")
                nc.tensor.transpose(psMT[:], Msb[:], eye_t[:])
                MTsb = smpool.tile([128, 128], F32, tag="MTsb")
                nc.scalar.copy(MTsb[:], psMT[:])
                psO = pmiscpool.tile([128, 128], F32, tag="psm")
                nc.tensor.matmul(psO[:], wvo_t[:], MTsb[:], start=True, stop=False)
                nc.tensor.matmul(psO[:], eye_t[:], FTsb[:], start=False, stop=True)
                Osb = smpool.tile([128, 128], F32, tag="Osb")
                nc.scalar.copy(Osb[:], psO[:])
                psOT = pmiscpool.tile([128, 128], F32, tag="psm")
                nc.tensor.transpose(psOT[:], Osb[:], eye_t[:])
                OTsb = smpool.tile([128, 128], F32, tag="OTsb")
                nc.scalar.copy(OTsb[:], psOT[:])
                nc.sync.dma_start(out_d[pair * 128:(pair + 1) * 128, :], OTsb[:])
    nc.compile()
    return nc


def kernel(**inputs):
    core_inputs = _host_prep(inputs)
    if "nc" not in _CACHE:
        _CACHE["nc"] = _build()
    nc = _CACHE["nc"]
    trace = (os.environ.get("BASS_TRACE_KERNEL", "") not in ("", "0")
             and _install_ntff_hook())
    res = bass_utils.run_bass_kernel_spmd(
        nc, core_inputs, list(range(BS)), trace=trace)
    _CACHE["last_results"] = res
    outs = [np.asarray(res.results[i]["out"], dtype=np.float32)
            for i in range(BS)]
    return np.stack(outs, axis=0)


# revision 24
# speedup vs baseline: 1.4526x; 1.0156x over previous
"""Trainium2 Bass kernel for triplane SO3 deformable attention.

Sharding: data-parallel over batch (8 batches -> 8 cores). Each core
processes 2048 queries against its own triplane.

Device pipeline per core (per pair of 64-sample blocks):
  - dma_gather fp16 x-pair rows (512B) for center + 8 rotated anchors
  - DVE multiply by broadcast geometric weights (bilinear lerp factors)
  - PE "pair matrix" matmuls reduce (y, corners, planes, anchors) into PSUM
  - wsum = feat @ W_wf on PE; anchors weighted by wsum (on-device dependency)
  - final out = mixed @ (W_v@W_o) + feat via PE, DMA out

Host side only shards, relayouts planes (fp16, x-pair duplicated rows),
computes gather indices / lerp weights, and folds projection weights.
"""

import os
import sys

import numpy as np

sys.path.insert(0, "/opt/trn_rl_repo")

import ml_dtypes  # noqa: E402

import concourse.bacc as bacc  # noqa: E402
import concourse.bass as bass  # noqa: E402
import concourse.mybir as mybir  # noqa: E402
import concourse.tile as tile  # noqa: E402
from concourse import bass_utils  # noqa: E402
from concourse.library_config import mlp  # noqa: E402


def _install_ntff_hook():
    """Provide antenv.axon_hooks (absent in this image) so that
    run_bass_kernel_spmd(trace=True) can capture NTFF profiles via the
    axon PJRT .so. Mirrors trn_agent_boot/trn_boot.py step 6."""
    import types

    if "antenv.axon_hooks" in sys.modules:
        return True
    try:
        sys.path.insert(0, "/root/.axon_site/trn_agent_boot")
        import trn_boot  # noqa: E402

        hook = trn_boot._ntff_profile_via_ctypes("/opt/axon/libaxon_pjrt.so")
        if hook is None:
            return False
        mod = types.ModuleType("antenv.axon_hooks")
        mod._hook = hook
        mod.get_axon_ntff_profile_hook = lambda: mod._hook
        mod.set_axon_ntff_profile_hook = lambda h: setattr(mod, "_hook", h)
        sys.modules["antenv.axon_hooks"] = mod
        return True
    except Exception:
        return False

BS, NS, NCP, NH, C, HID, R = 8, 2048, 8, 8, 128, 128, 128
NBLK = NS // 64          # 32 blocks of 64 samples
NPAIR = NBLK // 2        # 16 pairs (128 samples each)
NCHUNK = 16              # mix gathers: 16 chunks of 2 blocks per plane
F16 = mybir.dt.float16
F32 = mybir.dt.float32
I16 = mybir.dt.int16

_CACHE = {}


def _wrap_idx(flat):
    """int16 flat index list -> [128, N/16] wrapped+replicated dma_gather layout."""
    n = flat.shape[0]
    w = flat.reshape(n // 16, 16).T.astype(np.int16)  # [16, N/16], elem j at [j%16, j//16]
    return np.tile(w, (8, 1))


def _host_prep(inputs):
    q = np.asarray(inputs["query_pos"], dtype=np.float32)      # (8, 2048, 9)
    planes = [np.asarray(inputs[k], dtype=np.float32)
              for k in ("plane_xz", "plane_xy", "plane_yz")]    # (8, C, R, R)
    cp = np.asarray(inputs["control_points"], dtype=np.float32)  # (8, 3)
    W_v = np.asarray(inputs["W_v"], dtype=np.float32)
    W_w = np.asarray(inputs["W_w"], dtype=np.float32)
    W_o = np.asarray(inputs["W_o"], dtype=np.float32)

    # folded projections
    W_wf = W_w.reshape(C, NCP, NH).sum(axis=1)                  # (C, 8)
    W_vo = W_v @ W_o                                            # (C, C)

    # rotation 6d -> matrix (rows b1,b2,b3), all fp32
    a1, a2 = q[..., 3:6], q[..., 6:9]
    b1 = a1 / np.linalg.norm(a1, axis=-1, keepdims=True)
    b2 = a2 - np.sum(b1 * a2, axis=-1, keepdims=True) * b1
    b2 = b2 / np.linalg.norm(b2, axis=-1, keepdims=True)
    b3 = np.cross(b1, b2)
    rot = np.stack([b1, b2, b3], axis=-2)                       # (8, 2048, 3, 3)
    cpr = np.einsum("bnpd,gd->bngp", rot, cp).astype(np.float32)  # (8, 2048, 8, 3)
    pts = np.concatenate([q[:, :, None, :3], q[:, :, None, :3] + cpr], axis=2)
    # (8, 2048, 9, 3); anchor 0 = center

    coord_pairs = [(0, 2), (0, 1), (1, 2)]  # (x-dim, y-dim) for xz, xy, yz

    # static device constants
    pairc = np.eye(128, dtype=np.float16)  # fp16 identity (PSUM accumulate)
    eye = np.eye(128, dtype=np.float32)

    xs = np.minimum(np.arange(R) + 1, R - 1)
    ys = np.minimum(np.arange(R) + 1, R - 1)

    core_inputs = []
    for b in range(BS):
        im = {"pairc": pairc, "eye": eye,
              "wwf": W_wf.astype(np.float32), "wvo": W_vo.astype(np.float32)}
        idxf_all, idxm_all, w4_all = [], [], []
        for pi in range(3):
            P = planes[pi][b]                       # (C, R, R)
            PT = np.transpose(P, (1, 2, 0))         # (y, x, c)
            E = np.concatenate(
                [PT, PT[:, xs, :], PT[ys, :, :], PT[ys][:, xs, :]],
                axis=-1)                            # (R, R, 4C) 2x2 patches
            im[f"ep{pi}"] = np.ascontiguousarray(
                E.reshape(R * R, 4 * C)).astype(np.float16)

            cx, cy = coord_pairs[pi]
            u = pts[b, :, :, cx]                    # (2048, 9)
            v = pts[b, :, :, cy]
            x = np.clip(u, 0.0, 1.0).astype(np.float32) * np.float32(R - 1)
            y = np.clip(v, 0.0, 1.0).astype(np.float32) * np.float32(R - 1)
            x0 = np.floor(x); y0 = np.floor(y)
            fx = (x - x0).astype(np.float32); fy = (y - y0).astype(np.float32)
            x0i = x0.astype(np.int32); y0i = y0.astype(np.int32)
            idx = y0i * R + x0i                    # (2048, 9) patch row id

            # corner weights (2048, 9, 4) order (y0x0, y0x1, y1x0, y1x1)
            wy = np.stack([1.0 - fy, fy], axis=-1)
            wx = np.stack([1.0 - fx, fx], axis=-1)
            w4 = (wy[..., :, None] * wx[..., None, :]).reshape(NS, 9, 4)
            w4_all.append(w4.astype(np.float32))

            # feat indices: anchor 0, order (pair, s2) -> partition = s2
            af = idx[:, 0].reshape(NPAIR, 128).ravel()
            idxf_all.append(_wrap_idx(af))
            # mix indices: anchors 1..8, order (pair, a, s2)
            am = idx[:, 1:].reshape(NPAIR, 128, 8).transpose(0, 2, 1).ravel()
            idxm_all.append(_wrap_idx(am))

        im["idxf"] = np.concatenate(idxf_all, axis=1)   # [128, 3*128]
        im["idxm"] = np.concatenate(idxm_all, axis=1)   # [128, 3*1024]

        W4 = np.stack(w4_all, axis=2)                   # (2048, 9, 3, 4) [s,a,p,cor]
        gf = W4[:, 0].reshape(NPAIR, 128, 3, 4).transpose(1, 0, 2, 3)
        im["gwf"] = np.ascontiguousarray(
            gf.reshape(128, NPAIR * 12)).astype(np.float16)  # (pair, p, cor)
        gm = W4[:, 1:].reshape(NPAIR, 128, 8, 3, 4).transpose(1, 0, 3, 2, 4)
        im["gwm"] = np.ascontiguousarray(
            gm.reshape(128, NPAIR * 96)).astype(np.float16)  # (pair, p, a, cor)
        core_inputs.append(im)
    return core_inputs


def _build():
    nc = bacc.Bacc("TRN2", target_bir_lowering=False, num_swdge_queues=4)
    ep = [nc.dram_tensor(f"ep{p}", [R * R, 4 * C], F16, kind="ExternalInput")
          for p in range(3)]
    idxf_d = nc.dram_tensor("idxf", [128, 3 * 128], I16, kind="ExternalInput")
    idxm_d = nc.dram_tensor("idxm", [128, 3 * 1024], I16, kind="ExternalInput")
    gwf_d = nc.dram_tensor("gwf", [128, NPAIR * 12], F16, kind="ExternalInput")
    gwm_d = nc.dram_tensor("gwm", [128, NPAIR * 96], F16, kind="ExternalInput")
    pairc_d = nc.dram_tensor("pairc", [128, 128], F16, kind="ExternalInput")
    wwf_d = nc.dram_tensor("wwf", [C, NCP], F32, kind="ExternalInput")
    wvo_d = nc.dram_tensor("wvo", [C, C], F32, kind="ExternalInput")
    eye_d = nc.dram_tensor("eye", [128, 128], F32, kind="ExternalInput")
    out_d = nc.dram_tensor("out", [NS, C], F32, kind="ExternalOutput")

    with tile.TileContext(nc) as tc:
        with (
            tc.tile_pool(name="const", bufs=1) as cpool,
            tc.tile_pool(name="gf", bufs=1) as gfpool,
            tc.tile_pool(name="gm", bufs=3) as gmpool,
            tc.tile_pool(name="ym", bufs=3) as ympool,
            tc.tile_pool(name="sm", bufs=3) as smpool,
            tc.tile_pool(name="ft", bufs=NPAIR) as ftpool,
            tc.tile_pool(name="alp", bufs=NPAIR) as alpool,
            tc.tile_pool(name="pacc", bufs=2, space="PSUM") as paccpool,
            tc.tile_pool(name="pmix", bufs=2, space="PSUM") as pmixpool,
            tc.tile_pool(name="pmisc", bufs=4, space="PSUM") as pmiscpool,
        ):
            nc.gpsimd.load_library(mlp)

            def cload(name, dram, shape, dt):
                t = cpool.tile(shape, dt, tag=name)
                nc.sync.dma_start(t[:], dram[:])
                return t

            idxf_t = cload("idxf", idxf_d, [128, 3 * 128], I16)
            idxm_t = cload("idxm", idxm_d, [128, 3 * 1024], I16)
            gwf_t = cload("gwf", gwf_d, [128, NPAIR * 12], F16)
            gwm_t = cload("gwm", gwm_d, [128, NPAIR * 96], F16)
            ident_t = cload("pairc", pairc_d, [128, 128], F16)
            wwf_t = cload("wwf", wwf_d, [C, NCP], F32)
            wvo_t = cload("wvo", wvo_d, [C, C], F32)
            eye_t = cload("eye", eye_d, [128, 128], F32)

            # dma_gather crashes the exec unit above 1024 idx/call -> chunk
            qn = [0]

            def gather1k(dst, src_d, idx_t, col0, nidx):
                for h in range(nidx // 1024):
                    nc.gpsimd.dma_gather(
                        dst[:, h * 8:(h + 1) * 8, :], src_d[:],
                        idx_t[:, col0 + h * 64:col0 + (h + 1) * 64],
                        1024, 1024, 512, queue_num=qn[0] % 4)
                    qn[0] += 1

            # feat gathers: one patch row per sample: 2048 idx per plane
            gfeat = []
            for p in range(3):
                t = gfpool.tile([128, NPAIR, 512], F16, tag=f"gfe{p}")
                gather1k(t, ep[p], idxf_t, p * 128, 2048)
                gfeat.append(t)

            # mix gathers: per (pair, plane): 8 anchors * 128 samples = 1024
            gmix = {}
            for ch in range(NPAIR):
                for p in range(3):
                    t = gmpool.tile([128, 8, 512], F16, tag=f"gmx{p}")
                    gather1k(t, ep[p], idxm_t, p * 1024 + ch * 64, 1024)
                    gmix[(p, ch)] = t

            npair_run = int(os.environ.get("KPAIRS", str(NPAIR)))
            fts, als = {}, {}
            # ---- phase 1: feat -> wsum -> alpha for every pair (only
            # needs the small feat gathers, so it all runs early) ----
            for pair in range(npair_run):
                yfs = []
                for p in range(3):
                    yf = ympool.tile([128, 4, 128], F16, tag="yf")
                    in0 = gfeat[p][:, pair, :].rearrange(
                        "q (cor c) -> q cor c", cor=4)
                    gsl = gwf_t[:, pair * 12 + p * 4:pair * 12 + (p + 1) * 4]
                    in1 = gsl.unsqueeze(2).to_broadcast([128, 4, 128])
                    nc.vector.tensor_mul(yf[:], in0, in1)
                    yfs.append(yf)
                psF = paccpool.tile([128, 128], F32, tag="psF")
                n = 0
                for p in range(3):
                    for cor in range(4):
                        nc.tensor.matmul(
                            psF[:], ident_t[:], yfs[p][:, cor, :],
                            start=(n == 0), stop=(n == 11))
                        n += 1
                Fsb = smpool.tile([128, 128], F32, tag="Fsb")
                nc.scalar.copy(Fsb[:], psF[:])
                psFT = pmiscpool.tile([128, 128], F32, tag="psm")
                nc.tensor.transpose(psFT[:], Fsb[:], eye_t[:])
                FTsb = ftpool.tile([128, 128], F32, tag="FTsb")
                nc.scalar.copy(FTsb[:], psFT[:])
                psW = pmiscpool.tile([128, 128], F32, tag="psm")
                nc.tensor.matmul(psW[:NCP, :], wwf_t[:], FTsb[:],
                                 start=True, stop=True)
                Wsb = smpool.tile([NCP, 128], F32, tag="Wsb")
                nc.scalar.copy(Wsb[:], psW[:NCP, :])
                psWT = pmiscpool.tile([128, 128], F32, tag="psm")
                nc.tensor.transpose(psWT[:, :NCP], Wsb[:], eye_t[:NCP, :NCP])
                WTsb = smpool.tile([128, NCP], F16, tag="WTsb")
                nc.scalar.copy(WTsb[:], psWT[:, :NCP])
                al = alpool.tile([128, 96], F16, tag="al")
                in0 = gwm_t[:, pair * 96:(pair + 1) * 96].rearrange(
                    "q (p a cor) -> q p a cor", p=3, a=8)
                in1 = WTsb[:].unsqueeze(1).unsqueeze(3).to_broadcast(
                    [128, 3, 8, 4])
                nc.vector.tensor_mul(
                    al[:].rearrange("q (p a cor) -> q p a cor", p=3, a=8),
                    in0, in1)
                fts[pair], als[pair] = FTsb, al

            # ---- phase 2: mix multiply + reduce + projection (short
            # tail behind each pair's mix gather) ----
            for pair in range(npair_run):
                FTsb, al = fts[pair], als[pair]
                psM = pmixpool.tile([128, 128], F32, tag="psM")
                n = 0
                for p in range(3):
                    alx = ympool.tile([128, 32, 128], F16, tag="alx")
                    in1 = al[:, p * 32:(p + 1) * 32].unsqueeze(
                        2).to_broadcast([128, 32, 128])
                    nc.scalar.copy(alx[:], in1)
                    ym = ympool.tile([128, 32, 128], F16, tag="ym")
                    in0 = gmix[(p, pair)][:].rearrange(
                        "q a (cor c) -> q (a cor) c", cor=4)
                    nc.vector.tensor_mul(ym[:], in0, alx[:])
                    for k in range(32):
                        nc.tensor.matmul(
                            psM[:], ident_t[:], ym[:, k, :],
                            start=(n == 0), stop=(n == 95))
                        n += 1
                Msb = smpool.tile([128, 128], F32, tag="Msb")
                nc.scalar.copy(Msb[:], psM[:])
                psMT = pmiscpool.tile([128, 128], F32, tag="psm")
                nc.tensor.transpose(psMT[:], Msb[:], eye_t[:])
                MTsb = smpool.tile([128, 128], F32, tag="MTsb")
                nc.scalar.copy(MTsb[:], psMT[:])
                psO = pmiscpool.tile([128, 128], F32, tag="psm")
                nc.tensor.matmul(psO[:], wvo_t[:], MTsb[:], start=True, stop=False)
                nc.tensor.matmul(psO[:], eye_t[:], FTsb[:], start=False, stop=True)
                Osb = smpool.tile([128, 128], F32, tag="Osb")
                nc.scalar.copy(Osb[:], psO[:])
                psOT = pmiscpool.tile([128, 128], F32, tag="psm")
                nc.tensor.transpose(psOT[:], Osb[:], eye_t[:])
                OTsb = smpool.tile([128, 128], F32, tag="OTsb")
                nc.scalar.copy(OTsb[:], psOT[:])
                nc.sync.dma_start(out_d[pair * 128:(pair + 1) * 128, :], OTsb[:])
    nc.compile()
    return nc


def kernel(**inputs):
    core_inputs = _host_prep(inputs)
    if "nc" not in _CACHE:
        _CACHE["nc"] = _build()
    nc = _CACHE["nc"]
    trace = (os.environ.get("BASS_TRACE_KERNEL", "") not in ("", "0")
             and _install_ntff_hook())
    res = bass_utils.run_bass_kernel_spmd(
        nc, core_inputs, list(range(BS)), trace=trace)
    _CACHE["last_results"] = res
    outs = [np.asarray(res.results[i]["out"], dtype=np.float32)
            for i in range(BS)]
    return np.stack(outs, axis=0)
